# revision 19
# baseline (speedup 1.0000x reference)
"""Trainium2 Bass kernel v6 for nn_Attention (B=4, N=2048, D=1024, H=16, Hd=64).

Sharding: 8 cores = 4 batches x 2 head-groups. Core c: batch c//2, heads
[(c%2)*8, +8). Host sums the two partial y per batch + b_proj.

v6 on top of v5: (1) 2-3 EXPs/unit offloaded to a custom DVE polynomial-exp
op (quadratic+4 squarings) to cut the ScalarE 266us floor; (2) softmax
normalize fused into PV-accumulator evacuation (ut*rec from PSUM, kills the
uscr CAST pass); (3) denominator ones-matmuls issue as column-tile pairs so
they co-execute on the PE.

v5 schedule notes (still apply):
  - 16 units = (pair p, query-quarter qd) of 512 queries; unit order
    [p0q0 p0q1 p1q0 p1q1 p2q0 p3q0 p2q1 p3q1 p0q2 p1q2 p2q2 p3q2 ...q3]
    spreads the K/V fill demand and closes each quarter for its projection.
  - Per kt: double-buffered st tile [128,1024] f32 (A|B), both heads' score
    MMs adjacent (disjoint row groups -> concurrent), one [128,1024] EXP.
  - e tiles span 2 kt ([128,2048]) halving the DVE eacc op count.
  - PV lag 2; unit 0 defers its last 8 PVs + tail into unit 1 (v fills split
    across u0/u1); every unit's sums/normalize tail runs early in the next
    unit. Final-quarter projection is c-split so only the last pair's MM
    waits on the last normalize.
  - Input DMA triggers spread across 4 engine queues to unblock the head.
"""

import os
import sys
import types

import numpy as np

for _p in ("/opt/trn_rl_repo", "/root/.axon_site/_ro/trn_rl_repo"):
    if _p not in sys.path and os.path.isdir(_p):
        sys.path.append(_p)

import ml_dtypes  # noqa: E402

BF16 = ml_dtypes.bfloat16


def _install_ntff_shim():
    if "antenv.axon_hooks" in sys.modules:
        return
    mod = types.ModuleType("antenv.axon_hooks")
    mod._hook = None
    mod.set_axon_ntff_profile_hook = lambda h: setattr(mod, "_hook", h)
    mod.get_axon_ntff_profile_hook = lambda: mod._hook
    sys.modules["antenv.axon_hooks"] = mod
    try:
        import antenv

        antenv.axon_hooks = mod
    except ImportError:
        pass
    try:
        from trn_agent_boot.trn_boot import _ntff_profile_via_ctypes

        hook = _ntff_profile_via_ctypes("/opt/axon/libaxon_pjrt.so")
        if hook is not None:
            mod.set_axon_ntff_profile_hook(hook)
    except Exception:
        pass


_install_ntff_shim()

import concourse.bacc as bacc  # noqa: E402
import concourse.bass as bass  # noqa: E402
import concourse.tile as tile  # noqa: E402
from concourse import mybir  # noqa: E402
import concourse.bass_utils as bass_utils  # noqa: E402

bass_utils.upload_artifacts = lambda tmpdir: tmpdir

# ---- custom DVE op: exp(x*SCALE) ~= ((a*x + b)*x + c)^16 ------------------
# Quadratic Remez fit of e^u on u in [-0.15, 0.15] (u = x*SCALE/16), squared
# 4 times. Max rel err 2.3e-3 for |x*SCALE| <= 2.4 (scores are ~N(0, 0.33)).
import concourse.dve_ops as dve_ops  # noqa: E402
from concourse.dve_spec import Spec, Src0, C0, C1, C2, sq, lower as dve_lower  # noqa: E402
from concourse.dve_uop import DveOpSpec  # noqa: E402

EXPQ_A = 3.047468971427539e-05
EXPQ_B = 0.007834430888632058
EXPQ_C = 1.000015769339142


def _register_expq():
    if any(op.name == "EXPQ16_ANT" for op in dve_ops.OPS):
        return next(op for op in dve_ops.OPS if op.name == "EXPQ16_ANT")

    def _ref(in0, in1, s0, s1, imm2):
        x = in0.astype(np.float32)
        p = (x * s0 + s1) * x + imm2
        for _ in range(4):
            p = p * p
        return p

    spec = Spec(
        body=sq(sq(sq(sq((Src0 * C0 + C1) * Src0 + C2)))),
        reference=_ref,
    )
    row = dve_ops._CUSTOM_DVE_ROW_BASE + len(dve_ops.OPS)
    shas = {}
    for ver in ("v3", "v4"):
        try:
            ds = DveOpSpec(name="EXPQ16_ANT", opcode=row,
                           uops=dve_lower(spec, ver=ver), rd1_en=False)
            shas[ver] = ds.sha(ver)
        except Exception:
            pass
    op = dve_ops.DveOp("EXPQ16_ANT", spec, subdim=False, uops_sha=shas)
    dve_ops.OPS.append(op)
    dve_ops.CUSTOM_DVE_SPECS[op.name] = op.spec
    dve_ops._SUB_OPCODE_FOR_NAME[op.name] = row
    assert row < 0x20
    return op


EXPQ16 = _register_expq()

F32 = mybir.dt.float32
BF = mybir.dt.bfloat16
EXP = mybir.ActivationFunctionType.Exp
CPY = mybir.ActivationFunctionType.Copy

N_CORES = 8
NT = 2048
D = 1024
HD = 64
SCALE = HD**-0.5

# unit order: (pair, quarter)
ORDER = [
    (0, 0), (0, 1), (1, 0), (1, 1), (2, 0), (3, 0), (2, 1), (3, 1),
    (0, 2), (1, 2), (2, 2), (3, 2), (0, 3), (1, 3), (2, 3), (3, 3),
]
UIDX = {pq: i for i, pq in enumerate(ORDER)}

# kts whose EXP runs on the DVE (custom EXPQ16 op) instead of ScalarE;
# balances ScalarE (the busiest non-PE engine) against DVE slack.
DVE_KT = frozenset(
    int(t) for t in os.environ.get("KERNEL_DVE_KT", "6,12").split(",") if t != ""
)


def _body(tc: "tile.TileContext", ctx, y, xT, wqk, wv, wp):
    nc = tc.nc

    wpool = ctx.enter_context(tc.tile_pool(name="wpool", bufs=1))
    qkpool = ctx.enter_context(tc.tile_pool(name="qkpool", bufs=1))
    vpool = ctx.enter_context(tc.tile_pool(name="vpool", bufs=1))
    upool = ctx.enter_context(tc.tile_pool(name="upool", bufs=1))
    epool = ctx.enter_context(tc.tile_pool(name="epool", bufs=7))
    eaccpool = ctx.enter_context(tc.tile_pool(name="eaccpool", bufs=2))
    spool = ctx.enter_context(tc.tile_pool(name="spool", bufs=2))
    scrpool = ctx.enter_context(tc.tile_pool(name="scrpool", bufs=2))
    opool = ctx.enter_context(tc.tile_pool(name="opool", bufs=3))
    # PSUM (8 banks): st [128,1024] x2 (4) + ut [128,512] x2 (2) + fill [128,512] x2 (2)
    psb = ctx.enter_context(tc.tile_pool(name="psb", bufs=2, space="PSUM"))
    psu = ctx.enter_context(tc.tile_pool(name="psu", bufs=2, space="PSUM"))
    pfill = ctx.enter_context(tc.tile_pool(name="pfill", bufs=2, space="PSUM"))

    # ---- persistent SBUF tensors; DMA triggers spread over 4 queues -------
    xT_sb = [wpool.tile([128, NT], BF, tag=f"xT{i}", name=f"xT{i}") for i in range(8)]
    wqk_sb = [
        wpool.tile([128, 1024], BF, tag=f"wqk{i}", name=f"wqk{i}") for i in range(8)
    ]
    wv_sb = [wpool.tile([128, 512], BF, tag=f"wv{i}", name=f"wv{i}") for i in range(8)]
    wp_sb = [
        wpool.tile([128, 1024], BF, tag=f"wp{i}", name=f"wp{i}") for i in range(4)
    ]
    # DMA transfers in deadline-priority order, round-robined over the three
    # DMA-capable queues. The scalar queue gets only the lead transfers (its
    # later slots would delay the first EXPs).
    def _dma_plan():
        # P0: lead — x c0, Q f0-block, K f4-block (enough for the first unit)
        for d in range(8):
            yield xT_sb[d][:, 0:512], xT[d * 128 : (d + 1) * 128, 0:512], True
        for f in (0, 4):
            for d in range(8):
                yield (
                    wqk_sb[d][:, f * 128 : (f + 1) * 128],
                    wqk[d * 128 : (d + 1) * 128, f * 128 : (f + 1) * 128],
                    True,
                )
        # P1: V weights (v fills start at ~9us)
        for d in range(8):
            yield wv_sb[d][:, :], wv[d * 128 : (d + 1) * 128, :], True
        # P2+: x c1..c3, remaining wqk f-blocks by deadline, wp
        for d in range(8):
            yield xT_sb[d][:, 512:1024], xT[d * 128 : (d + 1) * 128, 512:1024], False
        for f in (5, 1):
            for d in range(8):
                yield (
                    wqk_sb[d][:, f * 128 : (f + 1) * 128],
                    wqk[d * 128 : (d + 1) * 128, f * 128 : (f + 1) * 128],
                    False,
                )
        for c in (2, 3):
            for d in range(8):
                yield (
                    xT_sb[d][:, c * 512 : (c + 1) * 512],
                    xT[d * 128 : (d + 1) * 128, c * 512 : (c + 1) * 512],
                    False,
                )
        for f in (6, 2, 7, 3):
            for d in range(8):
                yield (
                    wqk_sb[d][:, f * 128 : (f + 1) * 128],
                    wqk[d * 128 : (d + 1) * 128, f * 128 : (f + 1) * 128],
                    False,
                )
        for c in range(4):
            yield wp_sb[c][:, :], wp[c * 128 : (c + 1) * 128, :], False

    qi = [0, 0]  # lead rotates 3 queues, bulk rotates 2 (sync/gpsimd)
    for out_ap, in_ap, lead in _dma_plan():
        if lead:
            eng = (nc.sync, nc.scalar, nc.gpsimd)[qi[0] % 3]
            qi[0] += 1
        else:
            eng = (nc.sync, nc.gpsimd)[qi[1] % 2]
            qi[1] += 1
        eng.dma_start(out=out_ap, in_=in_ap)

    qkT = [qkpool.tile([128, NT], BF, tag=f"qkT{f}", name=f"qkT{f}") for f in range(8)]
    v_sb = [vpool.tile([128, 512], BF, tag=f"v{t}", name=f"v{t}") for t in range(16)]
    uhat = [upool.tile([128, NT], BF, tag=f"uh{p}", name=f"uh{p}") for p in range(4)]
    ones64 = wpool.tile([128, 64], BF, tag="ones64", name="ones64")
    nc.vector.memset(ones64, 1.0)

    # PE warmup: ~2us of dummy matmuls during the input-DMA lead so the HAM
    # clock gate is at 2.4 GHz when the first real fills execute (otherwise
    # the first ~3.4us of matmuls run at 1.2 GHz).
    warm_rhs = wpool.tile([128, 512], BF, tag="warm", name="warm_rhs")
    nc.vector.memset(warm_rhs, 0.0)
    warm_ps = pfill.tile([128, 512], F32, tag="pf", name="warm_ps")
    for _w in range(9):
        nc.tensor.matmul(warm_ps[0:64, :], ones64[:], warm_rhs[:, :],
                         start=True, stop=True)

    # ---- background fill units --------------------------------------------
    def qk_sub(f, ts2):
        ps = pfill.tile([128, 512], F32, tag="pf", name=f"qk_ps{f}_{ts2}")
        for d in range(8):
            nc.tensor.matmul(
                ps[:, :],
                wqk_sb[d][:, f * 128 : (f + 1) * 128],
                xT_sb[d][:, ts2 * 512 : (ts2 + 1) * 512],
                start=(d == 0),
                stop=(d == 7),
            )
        nc.vector.tensor_copy(out=qkT[f][:, ts2 * 512 : (ts2 + 1) * 512], in_=ps[:])

    def v_unit(t):
        ps = pfill.tile([128, 512], F32, tag="pf", name=f"v_ps{t}")
        for d in range(8):
            nc.tensor.matmul(
                ps[:, :],
                xT_sb[d][:, t * 128 : (t + 1) * 128],
                wv_sb[d][:, :],
                start=(d == 0),
                stop=(d == 7),
            )
        nc.vector.tensor_copy(out=v_sb[t], in_=ps[:])

    def proj_sub(qt, es, eng="v"):
        pj = pfill.tile([128, 512], F32, tag="pf", name=f"pj{qt}_{es}")
        for c in range(4):
            nc.tensor.matmul(
                pj[:, :],
                uhat[c][:, qt * 128 : (qt + 1) * 128],
                wp_sb[c][:, es * 512 : (es + 1) * 512],
                start=(c == 0),
                stop=(c == 3),
            )
        ot = opool.tile([128, 512], BF, tag="out", name=f"ot{qt}_{es}")
        with nc.allow_low_precision(reason="partial y output in bf16"):
            if eng == "s":
                nc.scalar.activation(out=ot, in_=pj[:], func=CPY)
            else:
                nc.vector.tensor_copy(out=ot, in_=pj[:])
        _oq = (nc.sync, nc.gpsimd)[(qt * 2 + es) % 2]
        _oq.dma_start(
            out=y[qt * 128 : (qt + 1) * 128, es * 512 : (es + 1) * 512], in_=ot
        )

    # ---- scores MMs: software-pipelined one kt AHEAD of the rest ----------
    def scores_mms(p, qd, kt):
        qsl = slice(qd * 512, (qd + 1) * 512)
        st = psb.tile([128, 1024], F32, tag="st", name=f"st{p}_{qd}_{kt}")
        nc.tensor.matmul(
            st[:, 0:512],
            qkT[4 + p][0:64, kt * 128 : (kt + 1) * 128],
            qkT[p][0:64, qsl],
            start=True,
            stop=True,
        )
        nc.tensor.matmul(
            st[:, 512:1024],
            qkT[4 + p][64:128, kt * 128 : (kt + 1) * 128],
            qkT[p][64:128, qsl],
            start=True,
            stop=True,
        )
        return st

    # ---- attention unit ----------------------------------------------------
    def attention_unit(p, qd, fills, carry, st0, nxt, defer_n=0, last=False):
        A, B = 2 * p, 2 * p + 1
        qsl = slice(qd * 512, (qd + 1) * 512)
        ut = psu.tile([128, 512], F32, tag="ut", name=f"ut{p}_{qd}")
        eacc = eaccpool.tile([128, 2048], BF, tag="eacc", name=f"eacc{p}_{qd}")
        es = []  # e tiles, one per 2 kt: [kt-even A|B (0:1024) | kt-odd A|B]

        def exp_of(kt, st):
            if kt % 2 == 0:
                e2 = epool.tile([128, 2048], BF, tag="e", name=f"e{p}_{qd}_{kt}")
                es.append(e2)
            out_ap = es[kt // 2][:, (kt % 2) * 1024 : (kt % 2) * 1024 + 1024]
            if kt in DVE_KT:
                nc.vector._custom_dve(
                    EXPQ16, out=out_ap, in0=st[:, :],
                    s0=EXPQ_A, s1=EXPQ_B, imm2=EXPQ_C,
                )
            else:
                nc.scalar.activation(
                    out=out_ap, in_=st[:, :], func=EXP, scale=SCALE,
                )

        def pv(kt):
            e2 = es[kt // 2]
            base = (kt % 2) * 1024
            nc.tensor.matmul(
                ut[0:64, :],
                v_sb[kt][:, A * 64 : (A + 1) * 64],
                e2[:, base : base + 512],
                start=(kt == 0),
                stop=(kt == 15),
                skip_group_check=True,
            )
            nc.tensor.matmul(
                ut[64:128, :],
                v_sb[kt][:, B * 64 : (B + 1) * 64],
                e2[:, base + 512 : base + 1024],
                start=(kt == 0),
                stop=(kt == 15),
                skip_group_check=True,
            )

        def eacc_step(j):
            if j == 1:
                nc.vector.tensor_add(out=eacc, in0=es[0], in1=es[1])
            elif j > 1:
                nc.vector.tensor_add(out=eacc, in0=eacc, in1=es[j])

        # in-loop PVs at lag 2: pv(kt-2) issued at kt
        pvp = {kt: [] for kt in range(16)}
        for _kt in range(2, 16):
            if _kt - 2 < 16 - defer_n:
                pvp[_kt].append(_kt - 2)
        st_next = None
        st_cur = st0
        for kt in range(16):
            # scores MMs + EXP are the pacing chain: priority 0 so the
            # scheduler always prefers them over slack work (fills/PVs).
            with tc.high_priority():
                exp_of(kt, st_cur)
                if kt < 15:
                    st_cur = scores_mms(p, qd, kt + 1)
                elif nxt is not None:
                    st_next = scores_mms(nxt[0], nxt[1], 0)
            # carry (prev unit's deferred PVs + tails) pops BEFORE fills:
            # quarter-close proj fills at this kt may read uhat written by
            # the carried tail2 normalize.
            if carry and kt >= 3:
                item = carry.pop(0)
                if item is not None:
                    item()
            for f in fills.get(kt, ()):
                f()
            if kt >= 3 and kt % 2 == 1 and (kt - 3) // 2 <= 6:
                eacc_step((kt - 3) // 2)
            for j in pvp[kt]:
                pv(j)
        while carry:
            item = carry.pop(0)
            if item is not None:
                item()
        if last:
            # split the final eacc add so the even-col half lands one EXP early
            nc.vector.tensor_add(
                out=eacc[:, 0:1024], in0=eacc[:, 0:1024], in1=es[7][:, 0:1024]
            )
            nc.vector.tensor_add(
                out=eacc[:, 1024:2048], in0=eacc[:, 1024:2048], in1=es[7][:, 1024:2048]
            )
        else:
            eacc_step(7)

        recs = []

        def tail1():
            # denominators: A = cols 0:512 + 1024:1536, B = 512:1024 + 1536:2048
            # issue order pairs (A,B) so col tiles (0,0)/(0,64) co-issue on PE
            sums_ps = pfill.tile([128, 512], F32, tag="pf", name=f"sums{p}_{qd}")
            nc.tensor.matmul(sums_ps[0:64, :], ones64[:], eacc[:, 0:512], start=True, stop=False, skip_group_check=True)
            nc.tensor.matmul(sums_ps[64:128, :], ones64[:], eacc[:, 512:1024], start=True, stop=False, skip_group_check=True)
            nc.tensor.matmul(sums_ps[0:64, :], ones64[:], eacc[:, 1024:1536], start=False, stop=True, skip_group_check=True)
            nc.tensor.matmul(sums_ps[64:128, :], ones64[:], eacc[:, 1536:2048], start=False, stop=True, skip_group_check=True)
            sums = spool.tile([128, 512], F32, tag="sums", name=f"sumss{p}_{qd}")
            nc.vector.tensor_copy(out=sums, in_=sums_ps[:])
            rows = []
            for hb, r0 in ((0, 0), (1, 64)):
                rsp = spool.tile([128, 4], F32, tag="rsp", name=f"rsp{p}_{qd}_{hb}")
                row = sums[r0 : r0 + 1, :].rearrange("p (a b) -> p a b", a=128)
                nc.gpsimd.dma_start(out=rsp[:], in_=row)
                rows.append(rsp)
            for hb, r0 in ((0, 0), (1, 64)):
                rspr = spool.tile([128, 4], BF, tag="rspr", name=f"rspr{p}_{qd}_{hb}")
                with nc.allow_low_precision(reason="softmax denom reciprocal in bf16"):
                    nc.vector.reciprocal(out=rspr[:], in_=rows[hb][:])
                rrow = spool.tile([1, 512], BF, tag="rrow", name=f"rrow{p}_{qd}_{hb}")
                nc.gpsimd.dma_start(
                    out=rrow[0:1, :].rearrange("p (a b) -> p a b", a=128), in_=rspr[:]
                )
                rec = scrpool.tile([128, 512], BF, tag=f"rec{hb}", name=f"rec{p}_{qd}_{hb}")
                nc.gpsimd.partition_broadcast(out_ap=rec[:, :], in_ap=rrow[0:1, :])
                recs.append(rec)

        def tail2():
            # fused normalize: uhat = ut * (1/denom), read PV accum from PSUM
            with nc.allow_low_precision(reason="softmax normalize in bf16"):
                for hb, r0 in ((0, 0), (1, 64)):
                    nc.vector.tensor_mul(
                        uhat[p][r0 : r0 + 64, qsl],
                        ut[r0 : r0 + 64, :],
                        recs[hb][r0 : r0 + 64, :],
                    )

        carry_out = []
        for kt2 in range(16 - defer_n, 16):
            carry_out.append(lambda kt2=kt2: pv(kt2))
        carry_out.append(tail1)
        carry_out.append(None)  # spacer: let the recip/broadcast chain land
        carry_out.append(tail2)
        return carry_out, st_next

    # ---- fill schedule -----------------------------------------------------
    def mk(fn, *a, **kw):
        return lambda: fn(*a, **kw)

    fills = {u: {} for u in range(16)}

    def add_fill(u, kt, f):
        fills[u].setdefault(kt, []).append(f)

    # v: first 8 in u0 (in-unit PVs 0..7 need them), rest spread in u1 such
    # that v(8+i) lands before the carried pv(8+i) fires at kt=3+i
    for t in range(8):
        add_fill(0, t, mk(v_unit, t))
    for t, kt in zip(range(8, 16), (0, 1, 2, 3, 6, 7, 8, 9)):
        add_fill(1, kt, mk(v_unit, t))
    # K features: pair 0 in u0; others: ts0 + Q early in the PRIOR unit
    # (kt 4/5 — clear of that unit's own early deadlines), ts1..3 inside the
    # pair's first unit before their kt deadline
    for ts in (1, 2, 3):
        add_fill(0, 4 * ts - 2, mk(qk_sub, 4, ts))
    for p in (1, 2, 3):
        u = UIDX[(p, 0)]
        add_fill(u - 1, 4 if u - 1 != 1 else 10, mk(qk_sub, 4 + p, 0))
        for ts in (1, 2, 3):
            add_fill(u, 4 * ts - 2, mk(qk_sub, 4 + p, ts))
    # Q features: in the prior unit
    for (p, qd), u in UIDX.items():
        if u == 0:
            continue
        add_fill(u - 1, 5 if u - 1 != 1 else 11, mk(qk_sub, p, qd))
    # proj: quarter qd closes at unit QCLOSE[qd]; weave its 8 subs into the
    # two units after (4 each); quarter 3 is handled in the tail.
    QCLOSE = {qd: max(UIDX[(p, qd)] for p in range(4)) for qd in range(4)}
    for qd in range(3):
        subs = [(qt, es) for qt in range(4 * qd, 4 * qd + 4) for es in range(2)]
        for i, (qt, es) in enumerate(subs):
            u = QCLOSE[qd] + 1 + i // 4
            add_fill(u, (7, 9, 11, 13)[i % 4], mk(proj_sub, qt, es))

    # ---- emit --------------------------------------------------------------
    qk_sub(0, 0)
    qk_sub(4, 0)
    carry = []
    with tc.high_priority():
        st_cur = scores_mms(0, 0, 0)
    for u, (p, qd) in enumerate(ORDER):
        nxt = ORDER[u + 1] if u + 1 < len(ORDER) else None
        carry, st_cur = attention_unit(
            p, qd, fills[u], carry, st_cur, nxt,
            defer_n=8 if u == 0 else 2, last=(u == 15)
        )
    for f in carry:
        if f is not None:
            f()
    # final-quarter projection, c-split: c0..c2 don't need the last normalize
    chunks = [(qt, es) for qt in range(12, 16) for es in range(2)]

    def proj_c3(qt, es, pj, eng):
        nc.tensor.matmul(
            pj[:, :],
            uhat[3][:, qt * 128 : (qt + 1) * 128],
            wp_sb[3][:, es * 512 : (es + 1) * 512],
            start=False,
            stop=True,
        )
        ot = opool.tile([128, 512], BF, tag="out", name=f"ot{qt}_{es}")
        with nc.allow_low_precision(reason="partial y output in bf16"):
            if eng == "s":
                nc.scalar.activation(out=ot, in_=pj[:], func=CPY)
            else:
                nc.vector.tensor_copy(out=ot, in_=pj[:])
        _oq = (nc.sync, nc.gpsimd)[(qt * 2 + es) % 2]
        _oq.dma_start(
            out=y[qt * 128 : (qt + 1) * 128, es * 512 : (es + 1) * 512], in_=ot
        )

    # chunk psum slots: 2x pfill + 2x st-halves + 2x ut = 6 concurrent; the
    # c012 burst keeps the PE warm while the last normalize chain completes.
    def chunk_slot(i):
        if i % 3 == 0:
            return pfill.tile([128, 512], F32, tag="pf", name=f"tp{i}")
        if i % 3 == 1:
            st = psb.tile([128, 1024], F32, tag="st", name=f"tp{i}")
            return st[:, 0:512]
        return psu.tile([128, 512], F32, tag="ut", name=f"tp{i}")

    open_pj = []
    ci = 0
    for i, (qt, es) in enumerate(chunks):
        if len(open_pj) == 6:
            oqt, oes, opj = open_pj.pop(0)
            proj_c3(oqt, oes, opj, "s" if ci % 2 == 0 else "v")
            ci += 1
        pj = chunk_slot(i)
        for c in range(3):
            nc.tensor.matmul(
                pj[:, :],
                uhat[c][:, qt * 128 : (qt + 1) * 128],
                wp_sb[c][:, es * 512 : (es + 1) * 512],
                start=(c == 0),
                stop=False,
            )
        open_pj.append((qt, es, pj))
    while open_pj:
        oqt, oes, opj = open_pj.pop(0)
        proj_c3(oqt, oes, opj, "s" if ci % 2 == 0 else "v")
        ci += 1


_NC_CACHE = {}


def _build_nc():
    if "nc" in _NC_CACHE:
        return _NC_CACHE["nc"]
    nc = bacc.Bacc("TRN2", target_bir_lowering=False, debug=False, num_devices=N_CORES)
    xT = nc.dram_tensor("xT", [D, NT], BF, kind="ExternalInput").ap()
    wqk = nc.dram_tensor("wqk", [D, 1024], BF, kind="ExternalInput").ap()
    wv = nc.dram_tensor("wv", [D, 512], BF, kind="ExternalInput").ap()
    wp = nc.dram_tensor("wp", [512, 1024], BF, kind="ExternalInput").ap()
    y = nc.dram_tensor("y", [NT, 1024], BF, kind="ExternalOutput").ap()
    from contextlib import ExitStack

    with tile.TileContext(nc) as tc, ExitStack() as ctx:
        _body(tc, ctx, y, xT, wqk, wv, wp)
    nc.compile()
    _NC_CACHE["nc"] = nc
    return nc


def _prepare_in_maps(x, W_qkv, W_proj):
    x = np.asarray(x, dtype=np.float32)
    W_qkv = np.asarray(W_qkv, dtype=np.float32)
    W_proj = np.asarray(W_proj, dtype=np.float32)
    in_maps = []
    for c in range(N_CORES):
        b, hg = divmod(c, 2)
        cs = slice(hg * 512, (hg + 1) * 512)
        xTc = np.ascontiguousarray(x[b].T).astype(BF16)
        wqk_m = np.ascontiguousarray(
            np.concatenate([W_qkv[:, 0:1024][:, cs], W_qkv[:, 1024:2048][:, cs]], axis=1)
        ).astype(BF16)
        wv_m = np.ascontiguousarray(W_qkv[:, 2048:3072][:, cs]).astype(BF16)
        wp_m = np.ascontiguousarray(W_proj[cs, :]).astype(BF16)
        in_maps.append({"xT": xTc, "wqk": wqk_m, "wv": wv_m, "wp": wp_m})
    return in_maps


def _run(x, W_qkv, W_proj, b_proj, trace=False):
    nc = _build_nc()
    in_maps = _prepare_in_maps(x, W_qkv, W_proj)
    res = bass_utils.run_bass_kernel_spmd(
        nc, in_maps, core_ids=list(range(N_CORES)), trace=trace
    )
    b_proj = np.asarray(b_proj, dtype=np.float32)
    y = np.empty((4, NT, D), dtype=np.float32)
    for b in range(4):
        y[b] = (
            res.results[2 * b]["y"].astype(np.float32)
            + res.results[2 * b + 1]["y"].astype(np.float32)
            + b_proj[None, :]
        )
    return y, res


def kernel(x, W_qkv, W_proj, b_proj):
    y, _ = _run(x, W_qkv, W_proj, b_proj, trace=False)
    return y



# revision 20
# speedup vs baseline: 1.0185x; 1.0185x over previous
"""Trainium2 Bass kernel v6 for nn_Attention (B=4, N=2048, D=1024, H=16, Hd=64).

Sharding: 8 cores = 4 batches x 2 head-groups. Core c: batch c//2, heads
[(c%2)*8, +8). Host sums the two partial y per batch + b_proj.

v6 on top of v5: (1) 2-3 EXPs/unit offloaded to a custom DVE polynomial-exp
op (quadratic+4 squarings) to cut the ScalarE 266us floor; (2) softmax
normalize fused into PV-accumulator evacuation (ut*rec from PSUM, kills the
uscr CAST pass); (3) denominator ones-matmuls issue as column-tile pairs so
they co-execute on the PE.

v5 schedule notes (still apply):
  - 16 units = (pair p, query-quarter qd) of 512 queries; unit order
    [p0q0 p0q1 p1q0 p1q1 p2q0 p3q0 p2q1 p3q1 p0q2 p1q2 p2q2 p3q2 ...q3]
    spreads the K/V fill demand and closes each quarter for its projection.
  - Per kt: double-buffered st tile [128,1024] f32 (A|B), both heads' score
    MMs adjacent (disjoint row groups -> concurrent), one [128,1024] EXP.
  - e tiles span 2 kt ([128,2048]) halving the DVE eacc op count.
  - PV lag 2; unit 0 defers its last 8 PVs + tail into unit 1 (v fills split
    across u0/u1); every unit's sums/normalize tail runs early in the next
    unit. Final-quarter projection is c-split so only the last pair's MM
    waits on the last normalize.
  - Input DMA triggers spread across 4 engine queues to unblock the head.
"""

import os
import sys
import types

import numpy as np

for _p in ("/opt/trn_rl_repo", "/root/.axon_site/_ro/trn_rl_repo"):
    if _p not in sys.path and os.path.isdir(_p):
        sys.path.append(_p)

import ml_dtypes  # noqa: E402

BF16 = ml_dtypes.bfloat16


def _install_ntff_shim():
    if "antenv.axon_hooks" in sys.modules:
        return
    mod = types.ModuleType("antenv.axon_hooks")
    mod._hook = None
    mod.set_axon_ntff_profile_hook = lambda h: setattr(mod, "_hook", h)
    mod.get_axon_ntff_profile_hook = lambda: mod._hook
    sys.modules["antenv.axon_hooks"] = mod
    try:
        import antenv

        antenv.axon_hooks = mod
    except ImportError:
        pass
    try:
        from trn_agent_boot.trn_boot import _ntff_profile_via_ctypes

        hook = _ntff_profile_via_ctypes("/opt/axon/libaxon_pjrt.so")
        if hook is not None:
            mod.set_axon_ntff_profile_hook(hook)
    except Exception:
        pass


_install_ntff_shim()

import concourse.bacc as bacc  # noqa: E402
import concourse.bass as bass  # noqa: E402
import concourse.tile as tile  # noqa: E402
from concourse import mybir  # noqa: E402
import concourse.bass_utils as bass_utils  # noqa: E402

bass_utils.upload_artifacts = lambda tmpdir: tmpdir

# ---- custom DVE op: exp(x*SCALE) ~= ((a*x + b)*x + c)^16 ------------------
# Quadratic Remez fit of e^u on u in [-0.15, 0.15] (u = x*SCALE/16), squared
# 4 times. Max rel err 2.3e-3 for |x*SCALE| <= 2.4 (scores are ~N(0, 0.33)).
import concourse.dve_ops as dve_ops  # noqa: E402
from concourse.dve_spec import Spec, Src0, C0, C1, C2, sq, lower as dve_lower  # noqa: E402
from concourse.dve_uop import DveOpSpec  # noqa: E402

EXPQ_A = 3.047468971427539e-05
EXPQ_B = 0.007834430888632058
EXPQ_C = 1.000015769339142


def _register_expq():
    if any(op.name == "EXPQ16_ANT" for op in dve_ops.OPS):
        return next(op for op in dve_ops.OPS if op.name == "EXPQ16_ANT")

    def _ref(in0, in1, s0, s1, imm2):
        x = in0.astype(np.float32)
        p = (x * s0 + s1) * x + imm2
        for _ in range(4):
            p = p * p
        return p

    spec = Spec(
        body=sq(sq(sq(sq((Src0 * C0 + C1) * Src0 + C2)))),
        reference=_ref,
    )
    row = dve_ops._CUSTOM_DVE_ROW_BASE + len(dve_ops.OPS)
    shas = {}
    for ver in ("v3", "v4"):
        try:
            ds = DveOpSpec(name="EXPQ16_ANT", opcode=row,
                           uops=dve_lower(spec, ver=ver), rd1_en=False)
            shas[ver] = ds.sha(ver)
        except Exception:
            pass
    op = dve_ops.DveOp("EXPQ16_ANT", spec, subdim=False, uops_sha=shas)
    dve_ops.OPS.append(op)
    dve_ops.CUSTOM_DVE_SPECS[op.name] = op.spec
    dve_ops._SUB_OPCODE_FOR_NAME[op.name] = row
    assert row < 0x20
    return op


EXPQ16 = _register_expq()

F32 = mybir.dt.float32
BF = mybir.dt.bfloat16
EXP = mybir.ActivationFunctionType.Exp
CPY = mybir.ActivationFunctionType.Copy

N_CORES = 8
NT = 2048
D = 1024
HD = 64
SCALE = HD**-0.5

# unit order: (pair, quarter)
ORDER = [
    (0, 0), (0, 1), (1, 0), (1, 1), (2, 0), (3, 0), (2, 1), (3, 1),
    (0, 2), (1, 2), (2, 2), (3, 2), (0, 3), (1, 3), (2, 3), (3, 3),
]
UIDX = {pq: i for i, pq in enumerate(ORDER)}

# kts whose EXP runs on the DVE (custom EXPQ16 op) instead of ScalarE;
# balances ScalarE (the busiest non-PE engine) against DVE slack.
DVE_KT = frozenset(
    int(t) for t in os.environ.get("KERNEL_DVE_KT", "6,12").split(",") if t != ""
)


def _body(tc: "tile.TileContext", ctx, y, xT, wqk, wv, wp):
    nc = tc.nc

    wpool = ctx.enter_context(tc.tile_pool(name="wpool", bufs=1))
    qkpool = ctx.enter_context(tc.tile_pool(name="qkpool", bufs=1))
    vpool = ctx.enter_context(tc.tile_pool(name="vpool", bufs=1))
    upool = ctx.enter_context(tc.tile_pool(name="upool", bufs=1))
    epool = ctx.enter_context(tc.tile_pool(name="epool", bufs=7))
    eaccpool = ctx.enter_context(tc.tile_pool(name="eaccpool", bufs=2))
    spool = ctx.enter_context(tc.tile_pool(name="spool", bufs=2))
    scrpool = ctx.enter_context(tc.tile_pool(name="scrpool", bufs=2))
    opool = ctx.enter_context(tc.tile_pool(name="opool", bufs=3))
    # PSUM (8 banks): st [128,1024] x2 (4) + ut [128,512] x2 (2) + fill [128,512] x2 (2)
    psb = ctx.enter_context(tc.tile_pool(name="psb", bufs=2, space="PSUM"))
    psu = ctx.enter_context(tc.tile_pool(name="psu", bufs=2, space="PSUM"))
    pfill = ctx.enter_context(tc.tile_pool(name="pfill", bufs=2, space="PSUM"))

    # ---- persistent SBUF tensors; DMA triggers spread over 4 queues -------
    xT_sb = [wpool.tile([128, NT], BF, tag=f"xT{i}", name=f"xT{i}") for i in range(8)]
    wqk_sb = [
        wpool.tile([128, 1024], BF, tag=f"wqk{i}", name=f"wqk{i}") for i in range(8)
    ]
    wv_sb = [wpool.tile([128, 512], BF, tag=f"wv{i}", name=f"wv{i}") for i in range(8)]
    wp_sb = [
        wpool.tile([128, 1024], BF, tag=f"wp{i}", name=f"wp{i}") for i in range(4)
    ]
    # DMA transfers in deadline-priority order, round-robined over the three
    # DMA-capable queues. The scalar queue gets only the lead transfers (its
    # later slots would delay the first EXPs).
    def _dma_plan():
        # P0: lead — x c0, Q f0-block, K f4-block (enough for the first unit)
        for d in range(8):
            yield xT_sb[d][:, 0:512], xT[d * 128 : (d + 1) * 128, 0:512], True
        for f in (0, 4):
            for d in range(8):
                yield (
                    wqk_sb[d][:, f * 128 : (f + 1) * 128],
                    wqk[d * 128 : (d + 1) * 128, f * 128 : (f + 1) * 128],
                    True,
                )
        # P1: V weights (v fills start at ~9us)
        for d in range(8):
            yield wv_sb[d][:, :], wv[d * 128 : (d + 1) * 128, :], True
        # P2+: x c1..c3, remaining wqk f-blocks by deadline, wp
        for d in range(8):
            yield xT_sb[d][:, 512:1024], xT[d * 128 : (d + 1) * 128, 512:1024], False
        for f in (5, 1):
            for d in range(8):
                yield (
                    wqk_sb[d][:, f * 128 : (f + 1) * 128],
                    wqk[d * 128 : (d + 1) * 128, f * 128 : (f + 1) * 128],
                    False,
                )
        for c in (2, 3):
            for d in range(8):
                yield (
                    xT_sb[d][:, c * 512 : (c + 1) * 512],
                    xT[d * 128 : (d + 1) * 128, c * 512 : (c + 1) * 512],
                    False,
                )
        for f in (6, 2, 7, 3):
            for d in range(8):
                yield (
                    wqk_sb[d][:, f * 128 : (f + 1) * 128],
                    wqk[d * 128 : (d + 1) * 128, f * 128 : (f + 1) * 128],
                    False,
                )
        for c in range(4):
            yield wp_sb[c][:, :], wp[c * 128 : (c + 1) * 128, :], False

    qi = [0, 0]  # lead rotates 3 queues, bulk rotates 2 (sync/gpsimd)
    for out_ap, in_ap, lead in _dma_plan():
        if lead:
            eng = (nc.sync, nc.scalar, nc.gpsimd)[qi[0] % 3]
            qi[0] += 1
        else:
            eng = (nc.sync, nc.gpsimd)[qi[1] % 2]
            qi[1] += 1
        eng.dma_start(out=out_ap, in_=in_ap)

    qkT = [qkpool.tile([128, NT], BF, tag=f"qkT{f}", name=f"qkT{f}") for f in range(8)]
    v_sb = [vpool.tile([128, 512], BF, tag=f"v{t}", name=f"v{t}") for t in range(16)]
    uhat = [upool.tile([128, NT], BF, tag=f"uh{p}", name=f"uh{p}") for p in range(4)]
    ones64 = wpool.tile([128, 64], BF, tag="ones64", name="ones64")
    nc.vector.memset(ones64, 1.0)

    # PE warmup: ~2us of dummy matmuls during the input-DMA lead so the HAM
    # clock gate is at 2.4 GHz when the first real fills execute (otherwise
    # the first ~3.4us of matmuls run at 1.2 GHz).
    warm_rhs = wpool.tile([128, 512], BF, tag="warm", name="warm_rhs")
    nc.vector.memset(warm_rhs, 0.0)
    warm_ps = pfill.tile([128, 512], F32, tag="pf", name="warm_ps")
    for _w in range(9):
        nc.tensor.matmul(warm_ps[0:64, :], ones64[:], warm_rhs[:, :],
                         start=True, stop=True)

    # ---- background fill units --------------------------------------------
    def qk_sub(f, ts2):
        ps = pfill.tile([128, 512], F32, tag="pf", name=f"qk_ps{f}_{ts2}")
        for d in range(8):
            nc.tensor.matmul(
                ps[:, :],
                wqk_sb[d][:, f * 128 : (f + 1) * 128],
                xT_sb[d][:, ts2 * 512 : (ts2 + 1) * 512],
                start=(d == 0),
                stop=(d == 7),
            )
        nc.vector.tensor_copy(out=qkT[f][:, ts2 * 512 : (ts2 + 1) * 512], in_=ps[:])

    def v_unit(t):
        ps = pfill.tile([128, 512], F32, tag="pf", name=f"v_ps{t}")
        for d in range(8):
            nc.tensor.matmul(
                ps[:, :],
                xT_sb[d][:, t * 128 : (t + 1) * 128],
                wv_sb[d][:, :],
                start=(d == 0),
                stop=(d == 7),
            )
        nc.vector.tensor_copy(out=v_sb[t], in_=ps[:])

    def proj_sub(qt, es, eng="v"):
        pj = pfill.tile([128, 512], F32, tag="pf", name=f"pj{qt}_{es}")
        for c in range(4):
            nc.tensor.matmul(
                pj[:, :],
                uhat[c][:, qt * 128 : (qt + 1) * 128],
                wp_sb[c][:, es * 512 : (es + 1) * 512],
                start=(c == 0),
                stop=(c == 3),
            )
        ot = opool.tile([128, 512], BF, tag="out", name=f"ot{qt}_{es}")
        with nc.allow_low_precision(reason="partial y output in bf16"):
            if eng == "s":
                nc.scalar.activation(out=ot, in_=pj[:], func=CPY)
            else:
                nc.vector.tensor_copy(out=ot, in_=pj[:])
        nc.sync.dma_start(
            out=y[qt * 128 : (qt + 1) * 128, es * 512 : (es + 1) * 512], in_=ot
        )

    # ---- scores MMs: software-pipelined one kt AHEAD of the rest ----------
    def scores_mms(p, qd, kt):
        qsl = slice(qd * 512, (qd + 1) * 512)
        st = psb.tile([128, 1024], F32, tag="st", name=f"st{p}_{qd}_{kt}")
        nc.tensor.matmul(
            st[:, 0:512],
            qkT[4 + p][0:64, kt * 128 : (kt + 1) * 128],
            qkT[p][0:64, qsl],
            start=True,
            stop=True,
        )
        nc.tensor.matmul(
            st[:, 512:1024],
            qkT[4 + p][64:128, kt * 128 : (kt + 1) * 128],
            qkT[p][64:128, qsl],
            start=True,
            stop=True,
        )
        return st

    # ---- attention unit ----------------------------------------------------
    def attention_unit(p, qd, fills, carry, st0, nxt, defer_n=0, last=False):
        A, B = 2 * p, 2 * p + 1
        qsl = slice(qd * 512, (qd + 1) * 512)
        ut = psu.tile([128, 512], F32, tag="ut", name=f"ut{p}_{qd}")
        eacc = eaccpool.tile([128, 2048], BF, tag="eacc", name=f"eacc{p}_{qd}")
        es = []  # e tiles, one per 2 kt: [kt-even A|B (0:1024) | kt-odd A|B]

        def exp_of(kt, st):
            if kt % 2 == 0:
                e2 = epool.tile([128, 2048], BF, tag="e", name=f"e{p}_{qd}_{kt}")
                es.append(e2)
            out_ap = es[kt // 2][:, (kt % 2) * 1024 : (kt % 2) * 1024 + 1024]
            if kt in DVE_KT:
                nc.vector._custom_dve(
                    EXPQ16, out=out_ap, in0=st[:, :],
                    s0=EXPQ_A, s1=EXPQ_B, imm2=EXPQ_C,
                )
            else:
                nc.scalar.activation(
                    out=out_ap, in_=st[:, :], func=EXP, scale=SCALE,
                )

        def pv(kt):
            e2 = es[kt // 2]
            base = (kt % 2) * 1024
            nc.tensor.matmul(
                ut[0:64, :],
                v_sb[kt][:, A * 64 : (A + 1) * 64],
                e2[:, base : base + 512],
                start=(kt == 0),
                stop=(kt == 15),
                skip_group_check=True,
            )
            nc.tensor.matmul(
                ut[64:128, :],
                v_sb[kt][:, B * 64 : (B + 1) * 64],
                e2[:, base + 512 : base + 1024],
                start=(kt == 0),
                stop=(kt == 15),
                skip_group_check=True,
            )

        def eacc_step(j):
            if j == 1:
                nc.vector.tensor_add(out=eacc, in0=es[0], in1=es[1])
            elif j > 1:
                nc.vector.tensor_add(out=eacc, in0=eacc, in1=es[j])

        # in-loop PVs at lag 2: pv(kt-2) issued at kt
        pvp = {kt: [] for kt in range(16)}
        for _kt in range(2, 16):
            if _kt - 2 < 16 - defer_n:
                pvp[_kt].append(_kt - 2)
        st_next = None
        st_cur = st0
        for kt in range(16):
            # scores MMs + EXP are the pacing chain: priority 0 so the
            # scheduler always prefers them over slack work (fills/PVs).
            with tc.high_priority():
                exp_of(kt, st_cur)
                if kt < 15:
                    st_cur = scores_mms(p, qd, kt + 1)
                elif nxt is not None:
                    st_next = scores_mms(nxt[0], nxt[1], 0)
            # carry (prev unit's deferred PVs + tails) pops BEFORE fills:
            # quarter-close proj fills at this kt may read uhat written by
            # the carried tail2 normalize.
            if carry and kt >= 3:
                item = carry.pop(0)
                if item is not None:
                    item()
            for f in fills.get(kt, ()):
                f()
            if kt >= 3 and kt % 2 == 1 and (kt - 3) // 2 <= 6:
                eacc_step((kt - 3) // 2)
            for j in pvp[kt]:
                pv(j)
        while carry:
            item = carry.pop(0)
            if item is not None:
                item()
        if last:
            # split the final eacc add so the even-col half lands one EXP early
            nc.vector.tensor_add(
                out=eacc[:, 0:1024], in0=eacc[:, 0:1024], in1=es[7][:, 0:1024]
            )
            nc.vector.tensor_add(
                out=eacc[:, 1024:2048], in0=eacc[:, 1024:2048], in1=es[7][:, 1024:2048]
            )
        else:
            eacc_step(7)

        recs = []

        def tail1():
            # denominators: A = cols 0:512 + 1024:1536, B = 512:1024 + 1536:2048
            # issue order pairs (A,B) so col tiles (0,0)/(0,64) co-issue on PE
            sums_ps = pfill.tile([128, 512], F32, tag="pf", name=f"sums{p}_{qd}")
            nc.tensor.matmul(sums_ps[0:64, :], ones64[:], eacc[:, 0:512], start=True, stop=False, skip_group_check=True)
            nc.tensor.matmul(sums_ps[64:128, :], ones64[:], eacc[:, 512:1024], start=True, stop=False, skip_group_check=True)
            nc.tensor.matmul(sums_ps[0:64, :], ones64[:], eacc[:, 1024:1536], start=False, stop=True, skip_group_check=True)
            nc.tensor.matmul(sums_ps[64:128, :], ones64[:], eacc[:, 1536:2048], start=False, stop=True, skip_group_check=True)
            sums = spool.tile([128, 512], F32, tag="sums", name=f"sumss{p}_{qd}")
            nc.vector.tensor_copy(out=sums, in_=sums_ps[:])
            rows = []
            for hb, r0 in ((0, 0), (1, 64)):
                rsp = spool.tile([128, 4], F32, tag="rsp", name=f"rsp{p}_{qd}_{hb}")
                row = sums[r0 : r0 + 1, :].rearrange("p (a b) -> p a b", a=128)
                nc.gpsimd.dma_start(out=rsp[:], in_=row)
                rows.append(rsp)
            for hb, r0 in ((0, 0), (1, 64)):
                rspr = spool.tile([128, 4], BF, tag="rspr", name=f"rspr{p}_{qd}_{hb}")
                with nc.allow_low_precision(reason="softmax denom reciprocal in bf16"):
                    nc.vector.reciprocal(out=rspr[:], in_=rows[hb][:])
                rrow = spool.tile([1, 512], BF, tag="rrow", name=f"rrow{p}_{qd}_{hb}")
                nc.gpsimd.dma_start(
                    out=rrow[0:1, :].rearrange("p (a b) -> p a b", a=128), in_=rspr[:]
                )
                rec = scrpool.tile([128, 512], BF, tag=f"rec{hb}", name=f"rec{p}_{qd}_{hb}")
                nc.gpsimd.partition_broadcast(out_ap=rec[:, :], in_ap=rrow[0:1, :])
                recs.append(rec)

        def tail2():
            # fused normalize: uhat = ut * (1/denom), read PV accum from PSUM
            with nc.allow_low_precision(reason="softmax normalize in bf16"):
                for hb, r0 in ((0, 0), (1, 64)):
                    nc.vector.tensor_mul(
                        uhat[p][r0 : r0 + 64, qsl],
                        ut[r0 : r0 + 64, :],
                        recs[hb][r0 : r0 + 64, :],
                    )

        carry_out = []
        for kt2 in range(16 - defer_n, 16):
            carry_out.append(lambda kt2=kt2: pv(kt2))
        carry_out.append(tail1)
        carry_out.append(None)  # spacer: let the recip/broadcast chain land
        carry_out.append(tail2)
        return carry_out, st_next

    # ---- fill schedule -----------------------------------------------------
    def mk(fn, *a, **kw):
        return lambda: fn(*a, **kw)

    fills = {u: {} for u in range(16)}

    def add_fill(u, kt, f):
        fills[u].setdefault(kt, []).append(f)

    # v: first 8 in u0 (in-unit PVs 0..7 need them), rest spread in u1 such
    # that v(8+i) lands before the carried pv(8+i) fires at kt=3+i
    for t in range(8):
        add_fill(0, t, mk(v_unit, t))
    for t, kt in zip(range(8, 16), (0, 1, 2, 3, 6, 7, 8, 9)):
        add_fill(1, kt, mk(v_unit, t))
    # K features: pair 0 in u0; others: ts0 + Q early in the PRIOR unit
    # (kt 4/5 — clear of that unit's own early deadlines), ts1..3 inside the
    # pair's first unit before their kt deadline
    for ts in (1, 2, 3):
        add_fill(0, 4 * ts - 2, mk(qk_sub, 4, ts))
    for p in (1, 2, 3):
        u = UIDX[(p, 0)]
        add_fill(u - 1, 4 if u - 1 != 1 else 10, mk(qk_sub, 4 + p, 0))
        for ts in (1, 2, 3):
            add_fill(u, 4 * ts - 2, mk(qk_sub, 4 + p, ts))
    # Q features: in the prior unit
    for (p, qd), u in UIDX.items():
        if u == 0:
            continue
        add_fill(u - 1, 5 if u - 1 != 1 else 11, mk(qk_sub, p, qd))
    # proj: quarter qd closes at unit QCLOSE[qd]; weave its 8 subs into the
    # two units after (4 each); quarter 3 is handled in the tail.
    QCLOSE = {qd: max(UIDX[(p, qd)] for p in range(4)) for qd in range(4)}
    for qd in range(3):
        subs = [(qt, es) for qt in range(4 * qd, 4 * qd + 4) for es in range(2)]
        for i, (qt, es) in enumerate(subs):
            u = QCLOSE[qd] + 1 + i // 4
            add_fill(u, (7, 9, 11, 13)[i % 4], mk(proj_sub, qt, es))

    # ---- emit --------------------------------------------------------------
    qk_sub(0, 0)
    qk_sub(4, 0)
    carry = []
    with tc.high_priority():
        st_cur = scores_mms(0, 0, 0)
    for u, (p, qd) in enumerate(ORDER):
        nxt = ORDER[u + 1] if u + 1 < len(ORDER) else None
        carry, st_cur = attention_unit(
            p, qd, fills[u], carry, st_cur, nxt,
            defer_n=8 if u == 0 else 2, last=(u == 15)
        )
    for f in carry:
        if f is not None:
            f()
    # final-quarter projection, c-split: c0..c2 don't need the last normalize
    chunks = [(qt, es) for qt in range(12, 16) for es in range(2)]

    def proj_c3(qt, es, pj, eng):
        nc.tensor.matmul(
            pj[:, :],
            uhat[3][:, qt * 128 : (qt + 1) * 128],
            wp_sb[3][:, es * 512 : (es + 1) * 512],
            start=False,
            stop=True,
        )
        ot = opool.tile([128, 512], BF, tag="out", name=f"ot{qt}_{es}")
        with nc.allow_low_precision(reason="partial y output in bf16"):
            if eng == "s":
                nc.scalar.activation(out=ot, in_=pj[:], func=CPY)
            else:
                nc.vector.tensor_copy(out=ot, in_=pj[:])
        nc.sync.dma_start(
            out=y[qt * 128 : (qt + 1) * 128, es * 512 : (es + 1) * 512], in_=ot
        )

    # chunk psum slots: 2x pfill + 2x st-halves + 2x ut = 6 concurrent; the
    # c012 burst keeps the PE warm while the last normalize chain completes.
    def chunk_slot(i):
        if i % 3 == 0:
            return pfill.tile([128, 512], F32, tag="pf", name=f"tp{i}")
        if i % 3 == 1:
            st = psb.tile([128, 1024], F32, tag="st", name=f"tp{i}")
            return st[:, 0:512]
        return psu.tile([128, 512], F32, tag="ut", name=f"tp{i}")

    open_pj = []
    ci = 0
    for i, (qt, es) in enumerate(chunks):
        if len(open_pj) == 6:
            oqt, oes, opj = open_pj.pop(0)
            proj_c3(oqt, oes, opj, "s" if ci % 2 == 0 else "v")
            ci += 1
        pj = chunk_slot(i)
        for c in range(3):
            nc.tensor.matmul(
                pj[:, :],
                uhat[c][:, qt * 128 : (qt + 1) * 128],
                wp_sb[c][:, es * 512 : (es + 1) * 512],
                start=(c == 0),
                stop=False,
            )
        open_pj.append((qt, es, pj))
    while open_pj:
        oqt, oes, opj = open_pj.pop(0)
        proj_c3(oqt, oes, opj, "s" if ci % 2 == 0 else "v")
        ci += 1


_NC_CACHE = {}


def _build_nc():
    if "nc" in _NC_CACHE:
        return _NC_CACHE["nc"]
    nc = bacc.Bacc("TRN2", target_bir_lowering=False, debug=False, num_devices=N_CORES)
    xT = nc.dram_tensor("xT", [D, NT], BF, kind="ExternalInput").ap()
    wqk = nc.dram_tensor("wqk", [D, 1024], BF, kind="ExternalInput").ap()
    wv = nc.dram_tensor("wv", [D, 512], BF, kind="ExternalInput").ap()
    wp = nc.dram_tensor("wp", [512, 1024], BF, kind="ExternalInput").ap()
    y = nc.dram_tensor("y", [NT, 1024], BF, kind="ExternalOutput").ap()
    from contextlib import ExitStack

    with tile.TileContext(nc) as tc, ExitStack() as ctx:
        _body(tc, ctx, y, xT, wqk, wv, wp)
    nc.compile()
    _NC_CACHE["nc"] = nc
    return nc


def _prepare_in_maps(x, W_qkv, W_proj):
    x = np.asarray(x, dtype=np.float32)
    W_qkv = np.asarray(W_qkv, dtype=np.float32)
    W_proj = np.asarray(W_proj, dtype=np.float32)
    in_maps = []
    for c in range(N_CORES):
        b, hg = divmod(c, 2)
        cs = slice(hg * 512, (hg + 1) * 512)
        xTc = np.ascontiguousarray(x[b].T).astype(BF16)
        wqk_m = np.ascontiguousarray(
            np.concatenate([W_qkv[:, 0:1024][:, cs], W_qkv[:, 1024:2048][:, cs]], axis=1)
        ).astype(BF16)
        wv_m = np.ascontiguousarray(W_qkv[:, 2048:3072][:, cs]).astype(BF16)
        wp_m = np.ascontiguousarray(W_proj[cs, :]).astype(BF16)
        in_maps.append({"xT": xTc, "wqk": wqk_m, "wv": wv_m, "wp": wp_m})
    return in_maps


def _run(x, W_qkv, W_proj, b_proj, trace=False):
    nc = _build_nc()
    in_maps = _prepare_in_maps(x, W_qkv, W_proj)
    res = bass_utils.run_bass_kernel_spmd(
        nc, in_maps, core_ids=list(range(N_CORES)), trace=trace
    )
    b_proj = np.asarray(b_proj, dtype=np.float32)
    y = np.empty((4, NT, D), dtype=np.float32)
    for b in range(4):
        y[b] = (
            res.results[2 * b]["y"].astype(np.float32)
            + res.results[2 * b + 1]["y"].astype(np.float32)
            + b_proj[None, :]
        )
    return y, res


def kernel(x, W_qkv, W_proj, b_proj):
    y, _ = _run(x, W_qkv, W_proj, b_proj, trace=False)
    return y



# revision 23
# speedup vs baseline: 1.0196x; 1.0011x over previous
"""Trainium2 Bass kernel v6 for nn_Attention (B=4, N=2048, D=1024, H=16, Hd=64).

Sharding: 8 cores = 4 batches x 2 head-groups. Core c: batch c//2, heads
[(c%2)*8, +8). Host sums the two partial y per batch + b_proj.

v6 on top of v5: (1) 2-3 EXPs/unit offloaded to a custom DVE polynomial-exp
op (quadratic+4 squarings) to cut the ScalarE 266us floor; (2) softmax
normalize fused into PV-accumulator evacuation (ut*rec from PSUM, kills the
uscr CAST pass); (3) denominator ones-matmuls issue as column-tile pairs so
they co-execute on the PE.

v5 schedule notes (still apply):
  - 16 units = (pair p, query-quarter qd) of 512 queries; unit order
    [p0q0 p0q1 p1q0 p1q1 p2q0 p3q0 p2q1 p3q1 p0q2 p1q2 p2q2 p3q2 ...q3]
    spreads the K/V fill demand and closes each quarter for its projection.
  - Per kt: double-buffered st tile [128,1024] f32 (A|B), both heads' score
    MMs adjacent (disjoint row groups -> concurrent), one [128,1024] EXP.
  - e tiles span 2 kt ([128,2048]) halving the DVE eacc op count.
  - PV lag 2; unit 0 defers its last 8 PVs + tail into unit 1 (v fills split
    across u0/u1); every unit's sums/normalize tail runs early in the next
    unit. Final-quarter projection is c-split so only the last pair's MM
    waits on the last normalize.
  - Input DMA triggers spread across 4 engine queues to unblock the head.
"""

import os
import sys
import types

import numpy as np

for _p in ("/opt/trn_rl_repo", "/root/.axon_site/_ro/trn_rl_repo"):
    if _p not in sys.path and os.path.isdir(_p):
        sys.path.append(_p)

import ml_dtypes  # noqa: E402

BF16 = ml_dtypes.bfloat16


def _install_ntff_shim():
    if "antenv.axon_hooks" in sys.modules:
        return
    mod = types.ModuleType("antenv.axon_hooks")
    mod._hook = None
    mod.set_axon_ntff_profile_hook = lambda h: setattr(mod, "_hook", h)
    mod.get_axon_ntff_profile_hook = lambda: mod._hook
    sys.modules["antenv.axon_hooks"] = mod
    try:
        import antenv

        antenv.axon_hooks = mod
    except ImportError:
        pass
    try:
        from trn_agent_boot.trn_boot import _ntff_profile_via_ctypes

        hook = _ntff_profile_via_ctypes("/opt/axon/libaxon_pjrt.so")
        if hook is not None:
            mod.set_axon_ntff_profile_hook(hook)
    except Exception:
        pass


_install_ntff_shim()

import concourse.bacc as bacc  # noqa: E402
import concourse.bass as bass  # noqa: E402
import concourse.tile as tile  # noqa: E402
from concourse import mybir  # noqa: E402
import concourse.bass_utils as bass_utils  # noqa: E402

bass_utils.upload_artifacts = lambda tmpdir: tmpdir

# ---- custom DVE op: exp(x*SCALE) ~= ((a*x + b)*x + c)^16 ------------------
# Quadratic Remez fit of e^u on u in [-0.15, 0.15] (u = x*SCALE/16), squared
# 4 times. Max rel err 2.3e-3 for |x*SCALE| <= 2.4 (scores are ~N(0, 0.33)).
import concourse.dve_ops as dve_ops  # noqa: E402
from concourse.dve_spec import Spec, Src0, C0, C1, C2, sq, lower as dve_lower  # noqa: E402
from concourse.dve_uop import DveOpSpec  # noqa: E402

EXPQ_A = 3.047468971427539e-05
EXPQ_B = 0.007834430888632058
EXPQ_C = 1.000015769339142


def _register_expq():
    if any(op.name == "EXPQ16_ANT" for op in dve_ops.OPS):
        return next(op for op in dve_ops.OPS if op.name == "EXPQ16_ANT")

    def _ref(in0, in1, s0, s1, imm2):
        x = in0.astype(np.float32)
        p = (x * s0 + s1) * x + imm2
        for _ in range(4):
            p = p * p
        return p

    spec = Spec(
        body=sq(sq(sq(sq((Src0 * C0 + C1) * Src0 + C2)))),
        reference=_ref,
    )
    row = dve_ops._CUSTOM_DVE_ROW_BASE + len(dve_ops.OPS)
    shas = {}
    for ver in ("v3", "v4"):
        try:
            ds = DveOpSpec(name="EXPQ16_ANT", opcode=row,
                           uops=dve_lower(spec, ver=ver), rd1_en=False)
            shas[ver] = ds.sha(ver)
        except Exception:
            pass
    op = dve_ops.DveOp("EXPQ16_ANT", spec, subdim=False, uops_sha=shas)
    dve_ops.OPS.append(op)
    dve_ops.CUSTOM_DVE_SPECS[op.name] = op.spec
    dve_ops._SUB_OPCODE_FOR_NAME[op.name] = row
    assert row < 0x20
    return op


EXPQ16 = _register_expq()

F32 = mybir.dt.float32
BF = mybir.dt.bfloat16
EXP = mybir.ActivationFunctionType.Exp
CPY = mybir.ActivationFunctionType.Copy

N_CORES = 8
NT = 2048
D = 1024
HD = 64
SCALE = HD**-0.5

# unit order: (pair, quarter)
ORDER = [
    (0, 0), (0, 1), (1, 0), (1, 1), (2, 0), (3, 0), (2, 1), (3, 1),
    (0, 2), (1, 2), (2, 2), (3, 2), (0, 3), (1, 3), (2, 3), (3, 3),
]
UIDX = {pq: i for i, pq in enumerate(ORDER)}

# kts whose EXP runs on the DVE (custom EXPQ16 op) instead of ScalarE;
# balances ScalarE (the busiest non-PE engine) against DVE slack.
DVE_KT = frozenset(
    int(t) for t in os.environ.get("KERNEL_DVE_KT", "6,12").split(",") if t != ""
)


def _body(tc: "tile.TileContext", ctx, y, xT, wqk, wv, wp):
    nc = tc.nc

    wpool = ctx.enter_context(tc.tile_pool(name="wpool", bufs=1))
    qkpool = ctx.enter_context(tc.tile_pool(name="qkpool", bufs=1))
    vpool = ctx.enter_context(tc.tile_pool(name="vpool", bufs=1))
    upool = ctx.enter_context(tc.tile_pool(name="upool", bufs=1))
    epool = ctx.enter_context(tc.tile_pool(name="epool", bufs=7))
    eaccpool = ctx.enter_context(tc.tile_pool(name="eaccpool", bufs=2))
    spool = ctx.enter_context(tc.tile_pool(name="spool", bufs=2))
    scrpool = ctx.enter_context(tc.tile_pool(name="scrpool", bufs=2))
    opool = ctx.enter_context(tc.tile_pool(name="opool", bufs=3))
    # PSUM (8 banks): st [128,1024] x2 (4) + ut [128,512] x2 (2) + fill [128,512] x2 (2)
    psb = ctx.enter_context(tc.tile_pool(name="psb", bufs=2, space="PSUM"))
    psu = ctx.enter_context(tc.tile_pool(name="psu", bufs=2, space="PSUM"))
    pfill = ctx.enter_context(tc.tile_pool(name="pfill", bufs=2, space="PSUM"))

    # ---- persistent SBUF tensors; DMA triggers spread over 4 queues -------
    xT_sb = [wpool.tile([128, NT], BF, tag=f"xT{i}", name=f"xT{i}") for i in range(8)]
    wqk_sb = [
        wpool.tile([128, 1024], BF, tag=f"wqk{i}", name=f"wqk{i}") for i in range(8)
    ]
    wv_sb = [wpool.tile([128, 512], BF, tag=f"wv{i}", name=f"wv{i}") for i in range(8)]
    wp_sb = [
        wpool.tile([128, 1024], BF, tag=f"wp{i}", name=f"wp{i}") for i in range(4)
    ]
    # DMA transfers in deadline-priority order, round-robined over the three
    # DMA-capable queues. The scalar queue gets only the lead transfers (its
    # later slots would delay the first EXPs).
    def _dma_plan():
        # P0: lead — x c0, Q f0-block, K f4-block (enough for the first unit)
        for d in range(8):
            yield xT_sb[d][:, 0:512], xT[d * 128 : (d + 1) * 128, 0:512], True
        for f in (0, 4):
            for d in range(8):
                yield (
                    wqk_sb[d][:, f * 128 : (f + 1) * 128],
                    wqk[d * 128 : (d + 1) * 128, f * 128 : (f + 1) * 128],
                    True,
                )
        # P1: V weights (v fills start at ~9us)
        for d in range(8):
            yield wv_sb[d][:, :], wv[d * 128 : (d + 1) * 128, :], True
        # P2+: x c1..c3, remaining wqk f-blocks by deadline, wp
        for d in range(8):
            yield xT_sb[d][:, 512:1024], xT[d * 128 : (d + 1) * 128, 512:1024], False
        for f in (5, 1):
            for d in range(8):
                yield (
                    wqk_sb[d][:, f * 128 : (f + 1) * 128],
                    wqk[d * 128 : (d + 1) * 128, f * 128 : (f + 1) * 128],
                    False,
                )
        for c in (2, 3):
            for d in range(8):
                yield (
                    xT_sb[d][:, c * 512 : (c + 1) * 512],
                    xT[d * 128 : (d + 1) * 128, c * 512 : (c + 1) * 512],
                    False,
                )
        for f in (6, 2, 7, 3):
            for d in range(8):
                yield (
                    wqk_sb[d][:, f * 128 : (f + 1) * 128],
                    wqk[d * 128 : (d + 1) * 128, f * 128 : (f + 1) * 128],
                    False,
                )
        for c in range(4):
            yield wp_sb[c][:, :], wp[c * 128 : (c + 1) * 128, :], False

    qi = [0, 0]  # lead rotates 3 queues, bulk rotates 2 (sync/gpsimd)
    for out_ap, in_ap, lead in _dma_plan():
        if lead:
            eng = (nc.sync, nc.scalar, nc.gpsimd)[qi[0] % 3]
            qi[0] += 1
        else:
            eng = (nc.sync, nc.gpsimd)[qi[1] % 2]
            qi[1] += 1
        eng.dma_start(out=out_ap, in_=in_ap)

    qkT = [qkpool.tile([128, NT], BF, tag=f"qkT{f}", name=f"qkT{f}") for f in range(8)]
    v_sb = [vpool.tile([128, 512], BF, tag=f"v{t}", name=f"v{t}") for t in range(16)]
    uhat = [upool.tile([128, NT], BF, tag=f"uh{p}", name=f"uh{p}") for p in range(4)]
    ones64 = wpool.tile([128, 64], BF, tag="ones64", name="ones64")
    nc.vector.memset(ones64, 1.0)

    # PE warmup: ~2us of dummy matmuls during the input-DMA lead so the HAM
    # clock gate is at 2.4 GHz when the first real fills execute (otherwise
    # the first ~3.4us of matmuls run at 1.2 GHz).
    warm_rhs = wpool.tile([128, 512], BF, tag="warm", name="warm_rhs")
    nc.vector.memset(warm_rhs, 0.0)
    warm_ps = pfill.tile([128, 512], F32, tag="pf", name="warm_ps")
    for _w in range(9):
        nc.tensor.matmul(warm_ps[0:64, :], ones64[:], warm_rhs[:, :],
                         start=True, stop=True)

    # ---- background fill units --------------------------------------------
    def qk_sub(f, ts2):
        ps = pfill.tile([128, 512], F32, tag="pf", name=f"qk_ps{f}_{ts2}")
        for d in range(8):
            nc.tensor.matmul(
                ps[:, :],
                wqk_sb[d][:, f * 128 : (f + 1) * 128],
                xT_sb[d][:, ts2 * 512 : (ts2 + 1) * 512],
                start=(d == 0),
                stop=(d == 7),
            )
        nc.vector.tensor_copy(out=qkT[f][:, ts2 * 512 : (ts2 + 1) * 512], in_=ps[:])

    def v_unit(t):
        ps = pfill.tile([128, 512], F32, tag="pf", name=f"v_ps{t}")
        for d in range(8):
            nc.tensor.matmul(
                ps[:, :],
                xT_sb[d][:, t * 128 : (t + 1) * 128],
                wv_sb[d][:, :],
                start=(d == 0),
                stop=(d == 7),
            )
        nc.vector.tensor_copy(out=v_sb[t], in_=ps[:])

    def proj_sub(qt, es, eng="v"):
        pj = pfill.tile([128, 512], F32, tag="pf", name=f"pj{qt}_{es}")
        for c in range(4):
            nc.tensor.matmul(
                pj[:, :],
                uhat[c][:, qt * 128 : (qt + 1) * 128],
                wp_sb[c][:, es * 512 : (es + 1) * 512],
                start=(c == 0),
                stop=(c == 3),
            )
        ot = opool.tile([128, 512], BF, tag="out", name=f"ot{qt}_{es}")
        with nc.allow_low_precision(reason="partial y output in bf16"):
            if eng == "s":
                nc.scalar.activation(out=ot, in_=pj[:], func=CPY)
            else:
                nc.vector.tensor_copy(out=ot, in_=pj[:])
        nc.sync.dma_start(
            out=y[qt * 128 : (qt + 1) * 128, es * 512 : (es + 1) * 512], in_=ot
        )

    # ---- scores MMs: software-pipelined one kt AHEAD of the rest ----------
    def scores_mms(p, qd, kt):
        qsl = slice(qd * 512, (qd + 1) * 512)
        st = psb.tile([128, 1024], F32, tag="st", name=f"st{p}_{qd}_{kt}")
        nc.tensor.matmul(
            st[:, 0:512],
            qkT[4 + p][0:64, kt * 128 : (kt + 1) * 128],
            qkT[p][0:64, qsl],
            start=True,
            stop=True,
        )
        nc.tensor.matmul(
            st[:, 512:1024],
            qkT[4 + p][64:128, kt * 128 : (kt + 1) * 128],
            qkT[p][64:128, qsl],
            start=True,
            stop=True,
        )
        return st

    # ---- attention unit ----------------------------------------------------
    def attention_unit(p, qd, fills, carry, st0, nxt, defer_n=0, last=False):
        A, B = 2 * p, 2 * p + 1
        qsl = slice(qd * 512, (qd + 1) * 512)
        ut = psu.tile([128, 512], F32, tag="ut", name=f"ut{p}_{qd}")
        eacc = eaccpool.tile([128, 2048], BF, tag="eacc", name=f"eacc{p}_{qd}")
        es = []  # e tiles, one per 2 kt: [kt-even A|B (0:1024) | kt-odd A|B]

        def exp_of(kt, st):
            if kt % 2 == 0:
                e2 = epool.tile([128, 2048], BF, tag="e", name=f"e{p}_{qd}_{kt}")
                es.append(e2)
            out_ap = es[kt // 2][:, (kt % 2) * 1024 : (kt % 2) * 1024 + 1024]
            if kt in DVE_KT:
                nc.vector._custom_dve(
                    EXPQ16, out=out_ap, in0=st[:, :],
                    s0=EXPQ_A, s1=EXPQ_B, imm2=EXPQ_C,
                )
            else:
                nc.scalar.activation(
                    out=out_ap, in_=st[:, :], func=EXP, scale=SCALE,
                )

        def pv(kt):
            e2 = es[kt // 2]
            base = (kt % 2) * 1024
            nc.tensor.matmul(
                ut[0:64, :],
                v_sb[kt][:, A * 64 : (A + 1) * 64],
                e2[:, base : base + 512],
                start=(kt == 0),
                stop=(kt == 15),
                skip_group_check=True,
            )
            nc.tensor.matmul(
                ut[64:128, :],
                v_sb[kt][:, B * 64 : (B + 1) * 64],
                e2[:, base + 512 : base + 1024],
                start=(kt == 0),
                stop=(kt == 15),
                skip_group_check=True,
            )

        def eacc_step(j):
            if j == 1:
                nc.vector.tensor_add(out=eacc, in0=es[0], in1=es[1])
            elif j > 1:
                nc.vector.tensor_add(out=eacc, in0=eacc, in1=es[j])

        # in-loop PVs at lag 2: pv(kt-2) issued at kt
        pvp = {kt: [] for kt in range(16)}
        for _kt in range(2, 16):
            if _kt - 2 < 16 - defer_n:
                pvp[_kt].append(_kt - 2)
        st_next = None
        st_cur = st0
        for kt in range(16):
            # scores MMs + EXP are the pacing chain: priority 0 so the
            # scheduler always prefers them over slack work (fills/PVs).
            with tc.high_priority():
                exp_of(kt, st_cur)
                if kt < 15:
                    st_cur = scores_mms(p, qd, kt + 1)
                elif nxt is not None:
                    st_next = scores_mms(nxt[0], nxt[1], 0)
            # carry (prev unit's deferred PVs + tails) pops BEFORE fills:
            # quarter-close proj fills at this kt may read uhat written by
            # the carried tail2 normalize.
            if carry and kt >= 3:
                item = carry.pop(0)
                if item is not None:
                    item()
            for f in fills.get(kt, ()):
                f()
            if kt >= 3 and kt % 2 == 1 and (kt - 3) // 2 <= 6:
                eacc_step((kt - 3) // 2)
            for j in pvp[kt]:
                pv(j)
        while carry:
            item = carry.pop(0)
            if item is not None:
                item()
        if last:
            # split the final eacc add so the even-col half lands one EXP early
            nc.vector.tensor_add(
                out=eacc[:, 0:1024], in0=eacc[:, 0:1024], in1=es[7][:, 0:1024]
            )
            nc.vector.tensor_add(
                out=eacc[:, 1024:2048], in0=eacc[:, 1024:2048], in1=es[7][:, 1024:2048]
            )
        else:
            eacc_step(7)

        recs = []

        def tail1():
            # denominators: A = cols 0:512 + 1024:1536, B = 512:1024 + 1536:2048
            # issue order pairs (A,B) so col tiles (0,0)/(0,64) co-issue on PE
            sums_ps = pfill.tile([128, 512], F32, tag="pf", name=f"sums{p}_{qd}")
            nc.tensor.matmul(sums_ps[0:64, :], ones64[:], eacc[:, 0:512], start=True, stop=False, skip_group_check=True)
            nc.tensor.matmul(sums_ps[64:128, :], ones64[:], eacc[:, 512:1024], start=True, stop=False, skip_group_check=True)
            nc.tensor.matmul(sums_ps[0:64, :], ones64[:], eacc[:, 1024:1536], start=False, stop=True, skip_group_check=True)
            nc.tensor.matmul(sums_ps[64:128, :], ones64[:], eacc[:, 1536:2048], start=False, stop=True, skip_group_check=True)
            sums = spool.tile([128, 512], F32, tag="sums", name=f"sumss{p}_{qd}")
            nc.vector.tensor_copy(out=sums, in_=sums_ps[:])
            rows = []
            for hb, r0 in ((0, 0), (1, 64)):
                rsp = spool.tile([128, 4], F32, tag="rsp", name=f"rsp{p}_{qd}_{hb}")
                row = sums[r0 : r0 + 1, :].rearrange("p (a b) -> p a b", a=128)
                nc.gpsimd.dma_start(out=rsp[:], in_=row)
                rows.append(rsp)
            for hb, r0 in ((0, 0), (1, 64)):
                rspr = spool.tile([128, 4], BF, tag="rspr", name=f"rspr{p}_{qd}_{hb}")
                with nc.allow_low_precision(reason="softmax denom reciprocal in bf16"):
                    nc.vector.reciprocal(out=rspr[:], in_=rows[hb][:])
                rrow = spool.tile([1, 512], BF, tag="rrow", name=f"rrow{p}_{qd}_{hb}")
                nc.gpsimd.dma_start(
                    out=rrow[0:1, :].rearrange("p (a b) -> p a b", a=128), in_=rspr[:]
                )
                rec = scrpool.tile([128, 512], BF, tag=f"rec{hb}", name=f"rec{p}_{qd}_{hb}")
                nc.gpsimd.partition_broadcast(out_ap=rec[:, :], in_ap=rrow[0:1, :])
                recs.append(rec)

        def tail2():
            # fused normalize: uhat = ut * (1/denom), read PV accum from PSUM
            with nc.allow_low_precision(reason="softmax normalize in bf16"):
                for hb, r0 in ((0, 0), (1, 64)):
                    nc.vector.tensor_mul(
                        uhat[p][r0 : r0 + 64, qsl],
                        ut[r0 : r0 + 64, :],
                        recs[hb][r0 : r0 + 64, :],
                    )

        carry_out = []
        for kt2 in range(16 - defer_n, 16):
            carry_out.append(lambda kt2=kt2: pv(kt2))
        carry_out.append(tail1)
        carry_out.append(None)  # spacer: let the recip/broadcast chain land
        carry_out.append(tail2)
        return carry_out, st_next

    # ---- fill schedule -----------------------------------------------------
    def mk(fn, *a, **kw):
        return lambda: fn(*a, **kw)

    fills = {u: {} for u in range(16)}

    def add_fill(u, kt, f):
        fills[u].setdefault(kt, []).append(f)

    # v: first 8 in u0 (in-unit PVs 0..7 need them), rest spread in u1 such
    # that v(8+i) lands before the carried pv(8+i) fires at kt=3+i
    for t in range(8):
        add_fill(0, t, mk(v_unit, t))
    for t, kt in zip(range(8, 16), (0, 1, 2, 3, 6, 7, 8, 9)):
        add_fill(1, kt, mk(v_unit, t))
    # K features: pair 0 in u0; others: ts0 + Q early in the PRIOR unit
    # (kt 4/5 — clear of that unit's own early deadlines), ts1..3 inside the
    # pair's first unit before their kt deadline
    for ts in (1, 2, 3):
        add_fill(0, 4 * ts - 2, mk(qk_sub, 4, ts))
    for p in (1, 2, 3):
        u = UIDX[(p, 0)]
        add_fill(u - 1, 4 if u - 1 != 1 else 10, mk(qk_sub, 4 + p, 0))
        for ts in (1, 2, 3):
            add_fill(u, 4 * ts - 2, mk(qk_sub, 4 + p, ts))
    # Q features: in the prior unit
    for (p, qd), u in UIDX.items():
        if u == 0:
            continue
        add_fill(u - 1, 5 if u - 1 != 1 else 11, mk(qk_sub, p, qd))
    # proj: quarter qd closes at unit QCLOSE[qd]; weave its 8 subs into the
    # two units after (4 each); quarter 3 is handled in the tail.
    QCLOSE = {qd: max(UIDX[(p, qd)] for p in range(4)) for qd in range(4)}
    for qd in range(3):
        subs = [(qt, es) for qt in range(4 * qd, 4 * qd + 4) for es in range(2)]
        for i, (qt, es) in enumerate(subs):
            u = QCLOSE[qd] + 1 + i // 4
            add_fill(u, (7, 9, 11, 13)[i % 4], mk(proj_sub, qt, es))

    # ---- emit --------------------------------------------------------------
    qk_sub(0, 0)
    qk_sub(4, 0)
    carry = []
    with tc.high_priority():
        st_cur = scores_mms(0, 0, 0)
    for u, (p, qd) in enumerate(ORDER):
        nxt = ORDER[u + 1] if u + 1 < len(ORDER) else None
        carry, st_cur = attention_unit(
            p, qd, fills[u], carry, st_cur, nxt,
            defer_n=8 if u == 0 else 2, last=(u == 15)
        )
    for f in carry:
        if f is not None:
            f()
    # final-quarter projection, c-split: c0..c2 don't need the last normalize
    chunks = [(qt, es) for qt in range(12, 16) for es in range(2)]

    def proj_c3(qt, es, pj, eng):
        nc.tensor.matmul(
            pj[:, :],
            uhat[3][:, qt * 128 : (qt + 1) * 128],
            wp_sb[3][:, es * 512 : (es + 1) * 512],
            start=False,
            stop=True,
        )
        ot = opool.tile([128, 512], BF, tag="out", name=f"ot{qt}_{es}")
        with nc.allow_low_precision(reason="partial y output in bf16"):
            if eng == "s":
                nc.scalar.activation(out=ot, in_=pj[:], func=CPY)
            else:
                nc.vector.tensor_copy(out=ot, in_=pj[:])
        nc.sync.dma_start(
            out=y[qt * 128 : (qt + 1) * 128, es * 512 : (es + 1) * 512], in_=ot
        )

    # chunk psum slots: 2x pfill + 2x st-halves + 2x ut = 6 concurrent; the
    # c012 burst keeps the PE warm while the last normalize chain completes.
    def chunk_slot(i):
        if i % 3 == 0:
            return pfill.tile([128, 512], F32, tag="pf", name=f"tp{i}")
        if i % 3 == 1:
            st = psb.tile([128, 1024], F32, tag="st", name=f"tp{i}")
            return st[:, 0:512]
        return psu.tile([128, 512], F32, tag="ut", name=f"tp{i}")

    open_pj = []
    ci = 0
    for i, (qt, es) in enumerate(chunks):
        if len(open_pj) == 6:
            oqt, oes, opj = open_pj.pop(0)
            proj_c3(oqt, oes, opj, "s" if ci % 2 == 0 else "v")
            ci += 1
        pj = chunk_slot(i)
        for c in range(3):
            nc.tensor.matmul(
                pj[:, :],
                uhat[c][:, qt * 128 : (qt + 1) * 128],
                wp_sb[c][:, es * 512 : (es + 1) * 512],
                start=(c == 0),
                stop=False,
            )
        open_pj.append((qt, es, pj))
    while open_pj:
        oqt, oes, opj = open_pj.pop(0)
        proj_c3(oqt, oes, opj, "s" if ci % 2 == 0 else "v")
        ci += 1


_NC_CACHE = {}


def _build_nc():
    if "nc" in _NC_CACHE:
        return _NC_CACHE["nc"]
    nc = bacc.Bacc("TRN2", target_bir_lowering=False, debug=False, num_devices=N_CORES)
    xT = nc.dram_tensor("xT", [D, NT], BF, kind="ExternalInput").ap()
    wqk = nc.dram_tensor("wqk", [D, 1024], BF, kind="ExternalInput").ap()
    wv = nc.dram_tensor("wv", [D, 512], BF, kind="ExternalInput").ap()
    wp = nc.dram_tensor("wp", [512, 1024], BF, kind="ExternalInput").ap()
    y = nc.dram_tensor("y", [NT, 1024], BF, kind="ExternalOutput").ap()
    from contextlib import ExitStack

    with tile.TileContext(nc) as tc, ExitStack() as ctx:
        _body(tc, ctx, y, xT, wqk, wv, wp)
    nc.compile()
    _NC_CACHE["nc"] = nc
    return nc


def _prepare_in_maps(x, W_qkv, W_proj):
    x = np.asarray(x, dtype=np.float32)
    W_qkv = np.asarray(W_qkv, dtype=np.float32)
    W_proj = np.asarray(W_proj, dtype=np.float32)
    in_maps = []
    for c in range(N_CORES):
        b, hg = divmod(c, 2)
        cs = slice(hg * 512, (hg + 1) * 512)
        xTc = np.ascontiguousarray(x[b].T).astype(BF16)
        wqk_m = np.ascontiguousarray(
            np.concatenate([W_qkv[:, 0:1024][:, cs], W_qkv[:, 1024:2048][:, cs]], axis=1)
        ).astype(BF16)
        wv_m = np.ascontiguousarray(W_qkv[:, 2048:3072][:, cs]).astype(BF16)
        wp_m = np.ascontiguousarray(W_proj[cs, :]).astype(BF16)
        in_maps.append({"xT": xTc, "wqk": wqk_m, "wv": wv_m, "wp": wp_m})
    return in_maps


def _run(x, W_qkv, W_proj, b_proj, trace=False):
    nc = _build_nc()
    in_maps = _prepare_in_maps(x, W_qkv, W_proj)
    res = bass_utils.run_bass_kernel_spmd(
        nc, in_maps, core_ids=list(range(N_CORES)), trace=trace
    )
    b_proj = np.asarray(b_proj, dtype=np.float32)
    y = np.empty((4, NT, D), dtype=np.float32)
    for b in range(4):
        y[b] = (
            res.results[2 * b]["y"].astype(np.float32)
            + res.results[2 * b + 1]["y"].astype(np.float32)
            + b_proj[None, :]
        )
    return y, res


def kernel(x, W_qkv, W_proj, b_proj):
    y, _ = _run(x, W_qkv, W_proj, b_proj, trace=False)
    return y

